# revision 5
# baseline (speedup 1.0000x reference)
"""Butterfly sparse-attention MLP kernel for 8 Trainium2 NeuronCores.

Computation (from the reference):
    attn = (w1.T @ w2.T) * sparse_mask          # [4096 s, 4096 t]
    y    = gelu(x @ attn + b2)                  # [8, 768, 4096]

sparse_mask is banded: mask[s, t] == 0 whenever |s - t| > 133.  Each core
owns a 512-wide t-block and needs only a 778-row s-window [t0-133, t0+645)
around it — the exact band, not rounded up to chunk alignment.  The window
is split into 6 full 128-row chunks plus a 10-row tail chunk.  Per
t-subtile of 128, the band covers window chunks q..q+3 (chunk q+3 only
10 rows deep, but mask zeros make the extra rows contribute nothing), so
phase B contracts over <=512 of s and phase A computes only the in-band
t-columns of each attn chunk.

Sharding: tensor-parallel over t (8 blocks of 512).  All per-core variation
is in the input data (windows are zero-padded at the edges), so one SPMD
BIR serves all 8 cores.

Matmul operands travel as fp16 (values are O(1), accumulation stays fp32
in PSUM) which halves HBM traffic; the mask travels as fp8 and is packed
to just the in-band columns.  Weights are host-shuffled into 6-8 KB DMA
rows; x/y move in 3 KB rows.  Streams are spread over the sync/scalar
HW-DGE queues plus the gpsimd SW-DGE queue: sync=w1+y/2,
scalar=w2+mask+b2+y/2, gpsimd=x.
"""

import numpy as np

B, T, D = 8, 768, 4096
N = B * T            # 6144 rows of x
NCORES = 8
TB = 512             # t-columns per core
P = 128
MARGIN = 133         # s-window extends this far before/after the t-block
SW = TB + 2 * MARGIN  # 778-row s-window
NCH = 7              # 6 full 128-row chunks + one 10-row tail
CH_ROWS = [128, 128, 128, 128, 128, 128, SW - 6 * P]   # last = 10
DCH = D // P         # 32 d-chunks (contraction of phase A)
NQ = TB // P         # 4 t-subtiles per core
GN = 1536            # n-group width in phase B
NG = N // GN         # 4 n-groups
MMN = 512            # moving-operand / PSUM-bank free-dim cap per matmul
BANDCH = 4           # s-chunks feeding one t-subtile (covers +-133 band)
W1PACK = 4           # w1 d-chunks packed per DMA row (6.1 KB descriptors)
W2PACK = 8           # w2T d-chunks packed per DMA row (8 KB descriptors)

_NC = None


def _band(j):
    """t-column range [lo, hi) of attn chunk j that phase B reads."""
    lo = P * max(0, j - (BANDCH - 1))
    hi = P * min(NQ - 1, j) + P
    return lo, hi


_MOFF = [0]
for _j in range(NCH):
    _lo, _hi = _band(_j)
    _MOFF.append(_MOFF[-1] + (_hi - _lo))
MW = _MOFF[-1]        # 2048 packed mask columns


def _build_module(mask_dt_name="float8e4"):
    from concourse import bacc, bass, mybir, tile
    from concourse.tile_rust import add_dep_helper

    f32 = mybir.dt.float32
    f16 = mybir.dt.float16
    mask_dt = getattr(mybir.dt, mask_dt_name)
    PSUM = bass.MemorySpace.PSUM

    nc = bacc.Bacc("TRN2", target_bir_lowering=False, debug=False)
    xT_d = nc.declare_dram_parameter("xT_s", [SW, N], f16, isOutput=False)
    w1_d = nc.declare_dram_parameter(
        "w1_s", [DCH // W1PACK, P, W1PACK * SW], f16, isOutput=False)
    w2T_d = nc.declare_dram_parameter(
        "w2T_s", [DCH // W2PACK, P, W2PACK * TB], f16, isOutput=False)
    mask_d = nc.declare_dram_parameter("mask_s", [P, MW], mask_dt,
                                       isOutput=False)
    b2_d = nc.declare_dram_parameter("b2c_s", [P, NQ], f32, isOutput=False)
    yT_d = nc.declare_dram_parameter("yT_s", [TB, N], f16, isOutput=True)

    with tile.TileContext(nc) as tc:
        with (
            tc.tile_pool(name="const", bufs=1) as cpool,
            tc.tile_pool(name="attn", bufs=1) as apool,
            tc.tile_pool(name="xp", bufs=NG * NCH) as xp,
            tc.tile_pool(name="yp", bufs=4) as yp,
        ):
            b2_t = cpool.tile([P, NQ], f32)
            nc.scalar.dma_start(b2_t[:], b2_d[:])
            m_t = cpool.tile([P, MW], mask_dt)
            nc.scalar.dma_start(m_t[:], mask_d[:])

            # ---- Phase A: attn[s, t] = (w1.T @ w2T) * mask on the band ----
            # Weights alternate between the sync/scalar HW-DGE queues so both
            # pull at full rate and the last d-chunk lands at the HBM-limited
            # ~30us mark instead of trailing on a single queue.
            attn_sb = []
            w1_insts = []
            with (
                tc.tile_pool(name="w1p", bufs=4) as w1p,
                tc.tile_pool(name="w2p", bufs=2) as w2p,
                tc.tile_pool(name="psA", bufs=1, space=PSUM) as psA,
            ):
                attn_ps = [
                    psA.tile([P, TB], f32, name=f"attn_ps{j}") for j in range(NCH)
                ]
                for bb in range(DCH // W2PACK):
                    w2_t = w2p.tile([P, W2PACK * TB], f16)
                    w2_eng = nc.sync if bb % 2 == 0 else nc.scalar
                    w2_eng.dma_start(w2_t[:], w2T_d[bb])
                    for hb in range(W2PACK // W1PACK):
                        pi = bb * (W2PACK // W1PACK) + hb
                        w1_t = w1p.tile([P, W1PACK * SW], f16)
                        w1_eng = nc.scalar if pi % 2 == 0 else nc.sync
                        w1_insts.append(w1_eng.dma_start(w1_t[:], w1_d[pi]))
                        for half in range(W1PACK):
                            k = bb * W2PACK + hb * W1PACK + half
                            w1sl = w1_t[:, half * SW:(half + 1) * SW]
                            w2sl = w2_t[:, (hb * W1PACK + half) * TB:
                                        (hb * W1PACK + half + 1) * TB]
                            for j in (3, 2, 4, 1, 5, 0, 6):
                                lo, hi = _band(j)
                                r = CH_ROWS[j]
                                nc.tensor.matmul(
                                    attn_ps[j][:r, lo:hi],
                                    w1sl[:, j * P:j * P + r],
                                    w2sl[:, lo:hi],
                                    start=(k == 0),
                                    stop=(k == DCH - 1),
                                )
                for j in range(NCH):
                    lo, hi = _band(j)
                    r = CH_ROWS[j]
                    a_t = apool.tile([P, TB], f16, name=f"attn_sb{j}")
                    nc.vector.tensor_mul(
                        a_t[:r, lo:hi], attn_ps[j][:r, lo:hi],
                        m_t[:r, _MOFF[j]:_MOFF[j + 1]],
                    )
                    attn_sb.append(a_t)

            # x rides the otherwise-idle gpsimd SW-DGE queue.  The first
            # four chunks (needed when phase B opens) flow immediately; the
            # rest are paced behind the 6th w1 load so early HBM bandwidth
            # goes to the weights, which gate all of phase B.
            x_t = {}
            for g in range(NG):
                for j in range(NCH):
                    r = CH_ROWS[j]
                    xt = xp.tile([P, GN], f16, name="x_t", tag="x_t")
                    xi = nc.gpsimd.dma_start(
                        xt[:r], xT_d[j * P:j * P + r, g * GN:(g + 1) * GN]
                    )
                    if g == 0 and j == BANDCH:
                        add_dep_helper(
                            xi.ins, w1_insts[5].ins,
                            sync=True, reason="pace x behind w1",
                        )
                    x_t[g, j] = xt

            # ---- Phase B: yT[t, n] = gelu(attn.T @ xT + b2) on the band ----
            with tc.tile_pool(name="psB", bufs=4, space=PSUM) as psB:
                st = 0
                for g in range(NG):
                    for q in range(NQ):
                        y_sb = yp.tile([P, GN], f16, name="y_sb", tag="y_sb")
                        for h in range(GN // MMN):
                            y_ps = psB.tile([P, MMN], f32, name="y_ps",
                                            tag="y_ps")
                            for c in range(BANDCH):
                                j = q + c
                                r = CH_ROWS[j]
                                nc.tensor.matmul(
                                    y_ps[:],
                                    attn_sb[j][:r, q * P:(q + 1) * P],
                                    x_t[g, j][:r, h * MMN:(h + 1) * MMN],
                                    start=(c == 0),
                                    stop=(c == BANDCH - 1),
                                )
                            nc.scalar.activation(
                                y_sb[:, h * MMN:(h + 1) * MMN],
                                y_ps[:],
                                mybir.ActivationFunctionType.Gelu,
                                bias=b2_t[:, q:q + 1],
                                scale=1.0,
                            )
                        st_eng = nc.sync if st % 2 == 0 else nc.scalar
                        st += 1
                        st_eng.dma_start(
                            yT_d[q * P:(q + 1) * P, g * GN:(g + 1) * GN],
                            y_sb[:],
                        )

    nc.compile()
    nc.finalize()
    return nc


def _get_nc():
    global _NC
    if _NC is None:
        try:
            _NC = _build_module("float8e4")
        except Exception:
            _NC = _build_module("float16")
    return _NC


def _mask_np_dtype():
    try:
        import ml_dtypes
        return np.dtype(ml_dtypes.float8_e4m3fn)
    except Exception:
        return None


def prepare_in_maps(x, w1, w2, b2, sparse_mask):
    x = np.asarray(x, dtype=np.float32)
    w1 = np.asarray(w1, dtype=np.float32)
    w2 = np.asarray(w2, dtype=np.float32)
    b2 = np.asarray(b2, dtype=np.float32)
    sparse_mask = np.asarray(sparse_mask, dtype=np.float32)

    xT = np.ascontiguousarray(x.reshape(N, D).T.astype(np.float16))   # [s, n]
    w2T = np.ascontiguousarray(w2.T.astype(np.float16))               # [d, t]

    # Zero-pad the s axis by MARGIN on both sides so every core's window is
    # a plain slice; mask zeros make the padded rows contribute nothing.
    xT_pad = np.zeros((D + 2 * MARGIN, N), dtype=np.float16)
    xT_pad[MARGIN:MARGIN + D] = xT
    w1_pad = np.zeros((D, D + 2 * MARGIN), dtype=np.float16)
    w1_pad[:, MARGIN:MARGIN + D] = w1.astype(np.float16)
    mask_pad = np.zeros((D + 2 * MARGIN, D), dtype=np.float16)
    mask_pad[MARGIN:MARGIN + D] = sparse_mask.astype(np.float16)

    mdt = _mask_np_dtype()
    in_maps = []
    for i in range(NCORES):
        s0 = i * TB           # window start in padded coords
        t0 = i * TB
        w1win = w1_pad[:, s0:s0 + SW]                     # [D, SW]
        w1_s = (w1win.reshape(DCH // W1PACK, W1PACK, P, SW)
                .transpose(0, 2, 1, 3)
                .reshape(DCH // W1PACK, P, W1PACK * SW))
        w2win = w2T[:, t0:t0 + TB]                        # [D, TB]
        w2_s = (w2win.reshape(DCH // W2PACK, W2PACK, P, TB)
                .transpose(0, 2, 1, 3)
                .reshape(DCH // W2PACK, P, W2PACK * TB))
        mwin = mask_pad[s0:s0 + SW, t0:t0 + TB]           # [SW, TB]
        m_s = np.zeros((P, MW), dtype=np.float16)
        for j in range(NCH):
            lo, hi = _band(j)
            r = CH_ROWS[j]
            m_s[:r, _MOFF[j]:_MOFF[j + 1]] = mwin[j * P:j * P + r, lo:hi]
        if mdt is not None:
            m_s = m_s.astype(mdt)
        in_maps.append({
            "xT_s": np.ascontiguousarray(xT_pad[s0:s0 + SW]),
            "w1_s": np.ascontiguousarray(w1_s),
            "w2T_s": np.ascontiguousarray(w2_s),
            "mask_s": np.ascontiguousarray(m_s),
            "b2c_s": np.ascontiguousarray(b2[t0:t0 + TB].reshape(NQ, P).T),
        })
    return in_maps


def assemble(results):
    out = np.empty((N, D), dtype=np.float32)
    for i in range(NCORES):
        out[:, i * TB:(i + 1) * TB] = results[i]["yT_s"].T.astype(np.float32)
    return out.reshape(B, T, D)


def _band_ok(sparse_mask):
    """The Bass kernel only computes attn where each core's 4-chunk window
    covers the mask; verify every mask nonzero falls inside that region."""
    s_idx, t_idx = np.nonzero(np.asarray(sparse_mask) != 0)
    if len(s_idx) == 0:
        return True
    w0 = (t_idx // TB) * TB - MARGIN          # per-core s-window start
    j = (s_idx - w0) // P                     # s-chunk within window
    q = (t_idx % TB) // P                     # t-subtile
    return bool(np.all((j >= q) & (j <= q + BANDCH - 1)
                       & (s_idx >= w0) & (s_idx < w0 + SW)))


def _reference_fallback(x, w1, w2, b2, sparse_mask):
    import jax
    import jax.numpy as jnp

    cpu = jax.devices("cpu")[0]
    with jax.default_device(cpu):
        attn = jnp.einsum("ds,td->st", jnp.asarray(w1), jnp.asarray(w2))
        attn = attn * jnp.asarray(sparse_mask)
        y = jnp.einsum("bds,st->bdt", jnp.asarray(x), attn) + jnp.asarray(b2)
        return np.asarray(jax.nn.gelu(y, approximate=False), dtype=np.float32)


def kernel(x, w1, w2, b2, sparse_mask):
    import time

    from concourse.bass_utils import run_bass_kernel_spmd

    if (np.shape(x) != (B, T, D) or np.shape(w1) != (D, D)
            or np.shape(w2) != (D, D) or np.shape(b2) != (D,)
            or np.shape(sparse_mask) != (D, D) or not _band_ok(sparse_mask)):
        return _reference_fallback(x, w1, w2, b2, sparse_mask)

    in_maps = prepare_in_maps(x, w1, w2, b2, sparse_mask)
    nc = _get_nc()
    last_err = None
    for attempt in range(3):
        try:
            res = run_bass_kernel_spmd(nc, in_maps, list(range(NCORES)))
            return assemble(res.results)
        except Exception as e:  # transient NRT/device errors: retry
            last_err = e
            time.sleep(2.0 * (attempt + 1))
    raise last_err


# revision 13
# speedup vs baseline: 1.0785x; 1.0785x over previous
"""Butterfly sparse-attention MLP kernel for 8 Trainium2 NeuronCores.

Computation (from the reference):
    attn = (w1.T @ w2.T) * sparse_mask          # [4096 s, 4096 t]
    y    = gelu(x @ attn + b2)                  # [8, 768, 4096]

sparse_mask is banded: mask[s, t] == 0 whenever |s - t| > 133.  Each core
owns a 512-wide t-block and needs only a 778-row s-window [t0-133, t0+645)
around it — the exact band, not rounded up to chunk alignment.  The window
is split into 6 full 128-row chunks plus a 10-row tail chunk.  Per
t-subtile of 128, the band covers window chunks q..q+3 (chunk q+3 only
10 rows deep, but mask zeros make the extra rows contribute nothing), so
phase B contracts over <=512 of s and phase A computes only the in-band
t-columns of each attn chunk.

Sharding: tensor-parallel over t (8 blocks of 512).  All per-core variation
is in the input data (windows are zero-padded at the edges), so one SPMD
BIR serves all 8 cores.

Matmul operands travel as fp16 (values are O(1), accumulation stays fp32
in PSUM) which halves HBM traffic; the mask travels as fp8 and is packed
to just the in-band columns.  Weights are host-shuffled into 6-8 KB DMA
rows; x/y move in 3 KB rows.  Streams are spread over the sync/scalar
HW-DGE queues plus the gpsimd SW-DGE queue: sync=w1+y/2,
scalar=w2+mask+b2+y/2, gpsimd=x.
"""

import numpy as np

B, T, D = 8, 768, 4096
N = B * T            # 6144 rows of x
NCORES = 8
TB = 512             # t-columns per core
P = 128
MARGIN = 133         # s-window extends this far before/after the t-block
SW = TB + 2 * MARGIN  # 778-row s-window
NCH = 7              # 6 full 128-row chunks + one 10-row tail
CH_ROWS = [128, 128, 128, 128, 128, 128, SW - 6 * P]   # last = 10
DCH = D // P         # 32 d-chunks (contraction of phase A)
NQ = TB // P         # 4 t-subtiles per core
GN = 1536            # n-group width in phase B
NG = N // GN         # 4 n-groups
MMN = 512            # moving-operand / PSUM-bank free-dim cap per matmul
BANDCH = 4           # s-chunks feeding one t-subtile (covers +-133 band)
W1PACK = 4           # w1 d-chunks packed per DMA row (6.1 KB descriptors)
W2PACK = 8           # w2T d-chunks packed per DMA row (8 KB descriptors)

_NC = None


def _band(j):
    """t-column range [lo, hi) of attn chunk j that phase B reads."""
    lo = P * max(0, j - (BANDCH - 1))
    hi = P * min(NQ - 1, j) + P
    return lo, hi


_MOFF = [0]
for _j in range(NCH):
    _lo, _hi = _band(_j)
    _MOFF.append(_MOFF[-1] + (_hi - _lo))
MW = _MOFF[-1]        # 2048 packed mask columns


def _build_module(mask_dt_name="float8e4"):
    from concourse import bacc, bass, mybir, tile
    from concourse.tile_rust import add_dep_helper

    f32 = mybir.dt.float32
    f16 = mybir.dt.float16
    mask_dt = getattr(mybir.dt, mask_dt_name)
    PSUM = bass.MemorySpace.PSUM

    nc = bacc.Bacc("TRN2", target_bir_lowering=False, debug=False)
    xT_d = nc.declare_dram_parameter("xT_s", [SW, N], f16, isOutput=False)
    w1_d = nc.declare_dram_parameter(
        "w1_s", [DCH // W1PACK, P, W1PACK * SW], f16, isOutput=False)
    w2T_d = nc.declare_dram_parameter(
        "w2T_s", [DCH // W2PACK, P, W2PACK * TB], f16, isOutput=False)
    mask_d = nc.declare_dram_parameter("mask_s", [P, MW], mask_dt,
                                       isOutput=False)
    b2_d = nc.declare_dram_parameter("b2c_s", [P, NQ], f32, isOutput=False)
    yT_d = nc.declare_dram_parameter("yT_s", [TB, N], f16, isOutput=True)

    with tile.TileContext(nc) as tc:
        with (
            tc.tile_pool(name="const", bufs=1) as cpool,
            tc.tile_pool(name="attn", bufs=1) as apool,
            tc.tile_pool(name="xp", bufs=16) as xp,
            tc.tile_pool(name="yp", bufs=4) as yp,
        ):
            b2_t = cpool.tile([P, NQ], f32)
            nc.scalar.dma_start(b2_t[:], b2_d[:])
            m_t = cpool.tile([P, MW], mask_dt)
            nc.scalar.dma_start(m_t[:], mask_d[:])

            # ---- Phase A: attn[s, t] = (w1.T @ w2T) * mask on the band ----
            # Weights alternate between the sync/scalar HW-DGE queues so both
            # pull at full rate and the last d-chunk lands at the HBM-limited
            # ~30us mark instead of trailing on a single queue.
            attn_sb = []
            w1_insts = []
            with (
                tc.tile_pool(name="w1p", bufs=DCH // W1PACK) as w1p,
                tc.tile_pool(name="w2p", bufs=DCH // W2PACK) as w2p,
                tc.tile_pool(name="psA", bufs=1, space=PSUM) as psA,
            ):
                attn_ps = [
                    psA.tile([P, TB], f32, name=f"attn_ps{j}") for j in range(NCH)
                ]
                for bb in range(DCH // W2PACK):
                    w2_t = w2p.tile([P, W2PACK * TB], f16)
                    w2_eng = nc.sync if bb % 2 == 0 else nc.scalar
                    w2_eng.dma_start(w2_t[:], w2T_d[bb])
                    for hb in range(W2PACK // W1PACK):
                        pi = bb * (W2PACK // W1PACK) + hb
                        w1_t = w1p.tile([P, W1PACK * SW], f16)
                        w1_eng = nc.scalar if pi % 2 == 0 else nc.sync
                        w1_insts.append(w1_eng.dma_start(w1_t[:], w1_d[pi]))
                        for half in range(W1PACK):
                            k = bb * W2PACK + hb * W1PACK + half
                            w1sl = w1_t[:, half * SW:(half + 1) * SW]
                            w2sl = w2_t[:, (hb * W1PACK + half) * TB:
                                        (hb * W1PACK + half + 1) * TB]
                            for j in (3, 2, 4, 1, 5, 0, 6):
                                lo, hi = _band(j)
                                r = CH_ROWS[j]
                                nc.tensor.matmul(
                                    attn_ps[j][:r, lo:hi],
                                    w1sl[:, j * P:j * P + r],
                                    w2sl[:, lo:hi],
                                    start=(k == 0),
                                    stop=(k == DCH - 1),
                                )
                for j in range(NCH):
                    lo, hi = _band(j)
                    r = CH_ROWS[j]
                    a_t = apool.tile([P, TB], f16, name=f"attn_sb{j}")
                    nc.vector.tensor_mul(
                        a_t[:r, lo:hi], attn_ps[j][:r, lo:hi],
                        m_t[:r, _MOFF[j]:_MOFF[j + 1]],
                    )
                    attn_sb.append(a_t)

            # x rides the otherwise-idle gpsimd SW-DGE queue.  The first
            # four chunks (needed when phase B opens) flow immediately; the
            # rest are paced behind the 6th w1 load so early HBM bandwidth
            # goes to the weights, which gate all of phase B.
            x_t = {}
            for g in range(NG):
                for j in range(NCH):
                    r = CH_ROWS[j]
                    xt = xp.tile([P, GN], f16, name="x_t", tag="x_t")
                    xi = nc.gpsimd.dma_start(
                        xt[:r], xT_d[j * P:j * P + r, g * GN:(g + 1) * GN]
                    )
                    if g == 0 and j == 0:
                        add_dep_helper(
                            xi.ins, w1_insts[4].ins,
                            sync=True, reason="pace x behind w1",
                        )
                    x_t[g, j] = xt

            # ---- Phase B: yT[t, n] = gelu(attn.T @ xT + b2) on the band ----
            with tc.tile_pool(name="psB", bufs=6, space=PSUM) as psB:
                st = 0
                NH = GN // MMN
                for g in range(NG):
                    for q in range(NQ):
                        y_sb = yp.tile([P, GN], f16, name="y_sb", tag="y_sb")
                        # c outer / h inner: the same attn stationary serves
                        # NH back-to-back matmuls, hiding the LDWEIGHTS.
                        y_pss = [
                            psB.tile([P, MMN], f32, name="y_ps", tag="y_ps")
                            for _ in range(NH)
                        ]
                        for c in range(BANDCH):
                            j = q + c
                            r = CH_ROWS[j]
                            for h in range(NH):
                                nc.tensor.matmul(
                                    y_pss[h][:],
                                    attn_sb[j][:r, q * P:(q + 1) * P],
                                    x_t[g, j][:r, h * MMN:(h + 1) * MMN],
                                    start=(c == 0),
                                    stop=(c == BANDCH - 1),
                                )
                        for h in range(NH):
                            nc.scalar.activation(
                                y_sb[:, h * MMN:(h + 1) * MMN],
                                y_pss[h][:],
                                mybir.ActivationFunctionType.Gelu,
                                bias=b2_t[:, q:q + 1],
                                scale=1.0,
                            )
                        st_eng = nc.sync if st % 2 == 0 else nc.scalar
                        st += 1
                        st_eng.dma_start(
                            yT_d[q * P:(q + 1) * P, g * GN:(g + 1) * GN],
                            y_sb[:],
                        )

    nc.compile()
    nc.finalize()
    return nc


_MASK_FP8 = None     # resolved on first _get_nc(): True -> fp8 mask path


def _get_nc():
    global _NC, _MASK_FP8
    if _NC is None:
        if _mask_np_dtype() is not None:
            try:
                _NC = _build_module("float8e4")
                _MASK_FP8 = True
            except Exception:
                _NC = _build_module("float16")
                _MASK_FP8 = False
        else:
            _NC = _build_module("float16")
            _MASK_FP8 = False
    return _NC


def _mask_np_dtype():
    try:
        import ml_dtypes
        return np.dtype(ml_dtypes.float8_e4m3fn)
    except Exception:
        return None


def prepare_in_maps(x, w1, w2, b2, sparse_mask):
    x = np.asarray(x, dtype=np.float32)
    w1 = np.asarray(w1, dtype=np.float32)
    w2 = np.asarray(w2, dtype=np.float32)
    b2 = np.asarray(b2, dtype=np.float32)
    sparse_mask = np.asarray(sparse_mask, dtype=np.float32)

    xT = np.ascontiguousarray(x.reshape(N, D).T.astype(np.float16))   # [s, n]
    w2T = np.ascontiguousarray(w2.T.astype(np.float16))               # [d, t]

    # Zero-pad the s axis by MARGIN on both sides so every core's window is
    # a plain slice; mask zeros make the padded rows contribute nothing.
    xT_pad = np.zeros((D + 2 * MARGIN, N), dtype=np.float16)
    xT_pad[MARGIN:MARGIN + D] = xT
    w1_pad = np.zeros((D, D + 2 * MARGIN), dtype=np.float16)
    w1_pad[:, MARGIN:MARGIN + D] = w1.astype(np.float16)
    mask_pad = np.zeros((D + 2 * MARGIN, D), dtype=np.float16)
    mask_pad[MARGIN:MARGIN + D] = sparse_mask.astype(np.float16)

    mdt = _mask_np_dtype()
    in_maps = []
    for i in range(NCORES):
        s0 = i * TB           # window start in padded coords
        t0 = i * TB
        w1win = w1_pad[:, s0:s0 + SW]                     # [D, SW]
        w1_s = (w1win.reshape(DCH // W1PACK, W1PACK, P, SW)
                .transpose(0, 2, 1, 3)
                .reshape(DCH // W1PACK, P, W1PACK * SW))
        w2win = w2T[:, t0:t0 + TB]                        # [D, TB]
        w2_s = (w2win.reshape(DCH // W2PACK, W2PACK, P, TB)
                .transpose(0, 2, 1, 3)
                .reshape(DCH // W2PACK, P, W2PACK * TB))
        mwin = mask_pad[s0:s0 + SW, t0:t0 + TB]           # [SW, TB]
        m_s = np.zeros((P, MW), dtype=np.float16)
        for j in range(NCH):
            lo, hi = _band(j)
            r = CH_ROWS[j]
            m_s[:r, _MOFF[j]:_MOFF[j + 1]] = mwin[j * P:j * P + r, lo:hi]
        if mdt is not None and _MASK_FP8:
            m_s = m_s.astype(mdt)
        in_maps.append({
            "xT_s": np.ascontiguousarray(xT_pad[s0:s0 + SW]),
            "w1_s": np.ascontiguousarray(w1_s),
            "w2T_s": np.ascontiguousarray(w2_s),
            "mask_s": np.ascontiguousarray(m_s),
            "b2c_s": np.ascontiguousarray(b2[t0:t0 + TB].reshape(NQ, P).T),
        })
    return in_maps


def assemble(results):
    out = np.empty((N, D), dtype=np.float32)
    for i in range(NCORES):
        out[:, i * TB:(i + 1) * TB] = results[i]["yT_s"].T.astype(np.float32)
    return out.reshape(B, T, D)


def _band_ok(sparse_mask):
    """The Bass kernel only computes attn where each core's 4-chunk window
    covers the mask; verify every mask nonzero falls inside that region."""
    s_idx, t_idx = np.nonzero(np.asarray(sparse_mask) != 0)
    if len(s_idx) == 0:
        return True
    w0 = (t_idx // TB) * TB - MARGIN          # per-core s-window start
    j = (s_idx - w0) // P                     # s-chunk within window
    q = (t_idx % TB) // P                     # t-subtile
    return bool(np.all((j >= q) & (j <= q + BANDCH - 1)
                       & (s_idx >= w0) & (s_idx < w0 + SW)))


def _reference_fallback(x, w1, w2, b2, sparse_mask):
    import jax
    import jax.numpy as jnp

    cpu = jax.devices("cpu")[0]
    with jax.default_device(cpu):
        attn = jnp.einsum("ds,td->st", jnp.asarray(w1), jnp.asarray(w2))
        attn = attn * jnp.asarray(sparse_mask)
        y = jnp.einsum("bds,st->bdt", jnp.asarray(x), attn) + jnp.asarray(b2)
        return np.asarray(jax.nn.gelu(y, approximate=False), dtype=np.float32)


def kernel(x, w1, w2, b2, sparse_mask):
    import time

    from concourse.bass_utils import run_bass_kernel_spmd

    if (np.shape(x) != (B, T, D) or np.shape(w1) != (D, D)
            or np.shape(w2) != (D, D) or np.shape(b2) != (D,)
            or np.shape(sparse_mask) != (D, D) or not _band_ok(sparse_mask)):
        return _reference_fallback(x, w1, w2, b2, sparse_mask)

    nc = _get_nc()           # resolves the mask dtype before prepare
    in_maps = prepare_in_maps(x, w1, w2, b2, sparse_mask)
    last_err = None
    for attempt in range(3):
        try:
            res = run_bass_kernel_spmd(nc, in_maps, list(range(NCORES)))
            return assemble(res.results)
        except Exception as e:  # transient NRT/device errors: retry
            last_err = e
            time.sleep(2.0 * (attempt + 1))
    raise last_err
